# revision 11
# baseline (speedup 1.0000x reference)
"""Trainium2 Bass kernel for ConfidenceCVXSelector.

Math: the reference builds A = fn fn^T (rank-2 Gram of row-normalized
(max_conf, dispersion) features), forms the normalized Laplacian
Ln = D~ - D^{-1/2} A D^{-1/2} and takes the Fiedler vector via dense eigh.

Because A is rank-2, Ln = I - G G^T with G = diag(dis) fn (dis = 1/sqrt(d),
d = fn @ s, s = sum_i fn_i). The non-trivial eigenvectors of Ln are G u for
eigenvectors u of the 2x2 matrix C = G^T G. s itself satisfies C s = s
(eigenvalue 1 <-> Ln eigenvalue 0), so the Fiedler vector is exactly
G u2 with u2 = perp(s) = (-S2, S1):

    fied_i = dis_i * (fn2_i * S1 - fn1_i * S2)

followed by the reference's sign canonicalization (flip so the largest-|.|
entry is positive) and min-max normalization — both invariant to the global
scale of fied, so no final renormalization is needed.

Per the sharding hint, the tiny reduced problem is solved redundantly:
the full 4096-element input is replicated to all 8 cores (each holds the
whole fn "shard" problem); core 0's output is returned. All compute is
O(N) elementwise + reductions on a single [128, 32] tile per core.
"""

import sys

if "/opt/trn_rl_repo" not in sys.path:
    sys.path.insert(0, "/opt/trn_rl_repo")

import numpy as np

import concourse.bass as bass
import concourse.bacc as bacc
import concourse.tile as tile
from concourse import bass_isa, mybir
from concourse.bass_utils import run_bass_kernel_spmd

F32 = mybir.dt.float32
AF = mybir.ActivationFunctionType
ALU = mybir.AluOpType

P, FREE = 128, 32  # 4096 = 128 partitions x 32 free
N_CORES = 8

_CACHE = {}


def _build_nc():
    nc = bacc.Bacc("TRN2", target_bir_lowering=False)
    x_d = nc.dram_tensor("x", [P, FREE], F32, kind="ExternalInput")
    y_d = nc.dram_tensor("y", [P, FREE], F32, kind="ExternalOutput")

    with tile.TileContext(nc) as tc:
        with tc.tile_pool(name="pool", bufs=1) as pool:
            X = pool.tile([P, FREE], F32, tag="X")
            AB = pool.tile([P, FREE], F32, tag="AB")
            E = pool.tile([P, FREE], F32, tag="E")
            DEN = pool.tile([P, FREE], F32, tag="DEN")
            U = pool.tile([P, FREE], F32, tag="U")
            U2 = pool.tile([P, FREE], F32, tag="U2")
            RT = pool.tile([P, FREE], F32, tag="RT")
            FN1 = pool.tile([P, FREE], F32, tag="FN1")
            FN2 = pool.tile([P, FREE], F32, tag="FN2")
            D1 = pool.tile([P, FREE], F32, tag="D1")
            D = pool.tile([P, FREE], F32, tag="D")
            SQD = pool.tile([P, FREE], F32, tag="SQD")
            DIS = pool.tile([P, FREE], F32, tag="DIS")
            W1 = pool.tile([P, FREE], F32, tag="W1")
            W = pool.tile([P, FREE], F32, tag="W")
            FIED = pool.tile([P, FREE], F32, tag="FIED")
            OUT = pool.tile([P, FREE], F32, tag="OUT")

            R = pool.tile([P, 2], F32, tag="R")       # row sums (fn1, fn2)
            SB = pool.tile([P, 2], F32, tag="SB")     # global sums bcast
            PACK = pool.tile([P, 2], F32, tag="PACK")  # (rowmax, -rowmin)
            GB = pool.tile([P, 2], F32, tag="GB")     # (a, -b) bcast

            T1 = pool.tile([P, 1], F32, tag="T1")
            G1 = pool.tile([P, 1], F32, tag="G1")
            SIG = pool.tile([P, 1], F32, tag="SIG")
            X1 = pool.tile([P, 1], F32, tag="X1")
            X2 = pool.tile([P, 1], F32, tag="X2")
            MP = pool.tile([P, 1], F32, tag="MP")
            RNG = pool.tile([P, 1], F32, tag="RNG")
            SCL = pool.tile([P, 1], F32, tag="SCL")
            SS = pool.tile([P, 1], F32, tag="SS")
            MS = pool.tile([P, 1], F32, tag="MS")
            WRM = pool.tile([1, 1], F32, tag="WRM")
            WRS = pool.tile([1, 1], F32, tag="WRS")

            # Load input
            nc.sync.dma_start(out=X[:, :], in_=x_d[:, :])

            # Warm the Exp activation table while the DMA is in flight.
            nc.vector.memset(WRM[:, :], 0.0)
            nc.scalar.activation(WRM[:, :], WRM[:, :], AF.Exp)

            # v = exp(-|x|)  (== (1-mc)/mc for mc = sigmoid(|x|));
            # |x| by clearing the sign bit (exact).
            nc.vector.tensor_scalar(
                AB.bitcast(mybir.dt.uint32)[:, :],
                X.bitcast(mybir.dt.uint32)[:, :],
                0x7FFFFFFF,
                None,
                op0=ALU.bitwise_and,
            )
            nc.scalar.activation(E[:, :], AB[:, :], AF.Exp, scale=-1.0)

            # Warm the Sqrt table right after the last Exp use (dep on E).
            nc.scalar.activation(WRS[:, :], E[0:1, 0:1], AF.Sqrt)

            # u = v * (1 + v); rows fn = (1, u)/sqrt(1+u^2)
            nc.vector.tensor_scalar(DEN[:, :], E[:, :], 1.0, None, op0=ALU.add)
            nc.vector.tensor_tensor(U[:, :], E[:, :], DEN[:, :], op=ALU.mult)
            nc.vector.tensor_tensor(U2[:, :], U[:, :], U[:, :], op=ALU.mult)
            nc.scalar.activation(RT[:, :], U2[:, :], AF.Sqrt, bias=1.0)
            nc.vector.reciprocal(FN1[:, :], RT[:, :])
            nc.vector.tensor_tensor(FN2[:, :], U[:, :], FN1[:, :], op=ALU.mult)
            nc.vector.tensor_reduce(
                R[:, 0:1], FN1[:, :], axis=mybir.AxisListType.X, op=ALU.add
            )
            nc.vector.tensor_reduce(
                R[:, 1:2], FN2[:, :], axis=mybir.AxisListType.X, op=ALU.add
            )

            # Global sums S = (sum fn1, sum fn2), broadcast to all partitions
            nc.gpsimd.partition_all_reduce(
                SB[:, :], R[:, :], channels=P, reduce_op=bass_isa.ReduceOp.add
            )

            # d = fn1*S1 + fn2*S2 ; dis = 1/sqrt(d)
            nc.vector.tensor_scalar(D1[:, :], FN1[:, :], SB[:, 0:1], None, op0=ALU.mult)
            nc.vector.scalar_tensor_tensor(
                D[:, :], in0=FN2[:, :], scalar=SB[:, 1:2], in1=D1[:, :],
                op0=ALU.mult, op1=ALU.add,
            )
            nc.scalar.activation(SQD[:, :], D[:, :], AF.Sqrt)
            nc.vector.reciprocal(DIS[:, :], SQD[:, :])

            # w = fn1*S2 - fn2*S1  (global sign of fied is canonicalized away)
            nc.vector.tensor_scalar(W1[:, :], FN2[:, :], SB[:, 0:1], None, op0=ALU.mult)
            nc.vector.scalar_tensor_tensor(
                W[:, :], in0=FN1[:, :], scalar=SB[:, 1:2], in1=W1[:, :],
                op0=ALU.mult, op1=ALU.subtract,
            )

            # fied = dis * w; row max and negated row min
            nc.vector.tensor_tensor(FIED[:, :], DIS[:, :], W[:, :], op=ALU.mult)
            nc.vector.tensor_reduce(
                PACK[:, 0:1], FIED[:, :], axis=mybir.AxisListType.X, op=ALU.max
            )
            nc.vector.tensor_reduce(
                PACK[:, 1:2], FIED[:, :], axis=mybir.AxisListType.X, op=ALU.min,
                negate=True,
            )

            # Global (a, -b) broadcast: a = max fied, b = min fied
            nc.gpsimd.partition_all_reduce(
                GB[:, :], PACK[:, :], channels=P, reduce_op=bass_isa.ReduceOp.max
            )

            # sigma = +1 if a + b >= 0 else -1 (matches argmax-|.| sign flip)
            nc.vector.tensor_tensor(T1[:, :], GB[:, 0:1], GB[:, 1:2], op=ALU.subtract)
            nc.vector.tensor_scalar(G1[:, :], T1[:, :], 0.0, None, op0=ALU.is_ge)
            nc.vector.tensor_scalar(
                SIG[:, :], G1[:, :], 2.0, 1.0, op0=ALU.mult, op1=ALU.subtract
            )
            # m' = min(sigma*a, sigma*b); range = a - b; out = (sigma*f - m')/range
            nc.vector.tensor_tensor(X1[:, :], SIG[:, :], GB[:, 0:1], op=ALU.mult)
            nc.vector.tensor_tensor(X2[:, :], SIG[:, :], GB[:, 1:2], op=ALU.mult)
            nc.vector.scalar_tensor_tensor(
                MP[:, :], in0=X2[:, :], scalar=-1.0, in1=X1[:, :],
                op0=ALU.mult, op1=ALU.min,
            )
            nc.vector.tensor_tensor(RNG[:, :], GB[:, 0:1], GB[:, 1:2], op=ALU.add)
            nc.vector.reciprocal(SCL[:, :], RNG[:, :])
            nc.vector.tensor_tensor(SS[:, :], SIG[:, :], SCL[:, :], op=ALU.mult)
            nc.vector.tensor_tensor(MS[:, :], MP[:, :], SCL[:, :], op=ALU.mult)
            nc.vector.tensor_scalar(
                OUT[:, :], FIED[:, :], SS[:, 0:1], MS[:, 0:1],
                op0=ALU.mult, op1=ALU.subtract,
            )

            nc.sync.dma_start(out=y_d[:, :], in_=OUT[:, :])

    nc.compile()
    return nc


def kernel(**inputs: np.ndarray) -> np.ndarray:
    x = np.ascontiguousarray(np.asarray(inputs["pred_logits"], dtype=np.float32))
    b, c, h, w = x.shape  # (1, 1, 64, 64)
    x2d = x.reshape(P, FREE)

    if "nc" not in _CACHE:
        _CACHE["nc"] = _build_nc()
    nc = _CACHE["nc"]

    in_maps = [{"x": x2d} for _ in range(N_CORES)]
    res = run_bass_kernel_spmd(nc, in_maps, core_ids=list(range(N_CORES)))
    out = np.asarray(res.results[0]["y"], dtype=np.float32)
    return out.reshape(b, c, h, w)


if __name__ == "__main__":
    rng = np.random.default_rng(0)
    x = rng.standard_normal((1, 1, 64, 64), dtype=np.float32)
    y = kernel(pred_logits=x)
    print("kernel out", y.shape, y.dtype, y.min(), y.max())
